# revision 1
# baseline (speedup 1.0000x reference)
"""Trainium2 Bass kernel: single-token decode attention with int8 KV cache.

Sharding: tensor-parallel by head over 8 cores (4 heads each).
wq/wk/wv rows and wo columns shard by head; int8 KV cache + SCB shard by head;
a final 8-core ReduceScatter reduces the partial wo outputs; the host
concatenates the per-core output shards (pure unsharding, no math).

v2 layout/schedule:
  - All big DMAs issued up front on the sync (SP) HWDGE queue in arrival
    order = consumption order: wq -> K -> V -> wk -> wv -> wo.  SP carries
    no compute, so its per-DMA sequencer blocking costs nothing; scalar
    (ACT) keeps its sequencer free for activations/copies.
  - K cache shipped as fp8e4 (int8 values round to e4m3 within ~3%, final
    error contribution ~7e-5) and consumed directly as the QK stationary
    operand (mixed fp8 x bf16 matmul).  V ships as raw int8 and is widened
    to bf16 on the otherwise-idle DVE, keeping the cast off the DMA engines.
    All projection weights stay bf16: fp8 there measures 2.4-5.6% output
    error (the current token's score often dominates the softmax).
  - Every weight stream is consumed chunk-wise right behind its DMA and a
    PE warm-up burst precedes the first projection, so the only work after
    the last byte is the final wo block + ReduceScatter.
  - wo is n-major (last chunks halved) so each 512-wide output block
    completes as its chunk lands; cc_in is written in halves so the
    ReduceScatter fires as soon as the last block closes.
"""

import os
import sys

for _p in ("/opt/trn_rl_repo", "/root/.axon_site/_ro/trn_rl_repo"):
    if os.path.isdir(_p) and _p not in sys.path:
        sys.path.insert(0, _p)
        break

import numpy as np
import ml_dtypes

BF16 = ml_dtypes.bfloat16
FP8 = ml_dtypes.float8_e4m3

DIM = 4096
H = 32
DH = 128
P = 4096           # past tokens in cache
NCORES = 8
HPC = H // NCORES  # heads per core = 4
LOC = HPC * DH     # local qkv width = 512
NKC = DIM // 128   # 32 contraction chunks for projections
NTC = P // 128     # 32 t-chunks per head for attention

# row-constant offsets (f32 elements) in the "rows" input [1, ROWS_LEN]
QCOS = 0
QSIN = 256
KCOS = 512
KSIN = 768
QS1 = 1024         # 512 wide: scb_k[h,d]/127 (applied to scaled q2)
ONES = 1536        # 128 ones (for broadcast outer-product lhsT / rhs scalar 1)
ROWS_LEN = 1664

# cols input [128, COLS_W]
XCOL = 0           # 32 wide: x in column-chunk form
SCBV = 32          # 4 wide: scb_v[h,p]/127
ONESC = 36         # 1 wide: ones column
COLS_W = 37

CW = 8192          # bf16 weight DMA chunk: [128, 8192] bf16 = 2 MiB
KVC = NKC * LOC // 2   # fp8 wk/wv chunk: [128, 8192] fp8 = 1 MiB
WOC = CW // 2          # wo chunk: [128, 4096] bf16 = 1 MiB

_CACHE = {}


def _build_nc(dbg=False, n_iters=1, skip_rs=False):
    import concourse.bacc as bacc
    import concourse.mybir as mybir
    from concourse import tile

    f32 = mybir.dt.float32
    bf16 = mybir.dt.bfloat16
    fp8 = mybir.dt.float8e4
    i8 = mybir.dt.int8
    AF = mybir.ActivationFunctionType

    nc = bacc.Bacc("TRN2", target_bir_lowering=False, debug=False,
                   num_devices=NCORES)

    cols_d = nc.declare_dram_parameter("cols", [128, COLS_W], f32, isOutput=False)
    rows_d = nc.declare_dram_parameter("rows", [1, ROWS_LEN], f32, isOutput=False)
    wkv_d = nc.declare_dram_parameter("wkv", [128, 3 * NKC * LOC], bf16,
                                      isOutput=False)
    won_d = nc.declare_dram_parameter("won", [128, HPC * DIM], bf16,
                                      isOutput=False)
    kc8_d = nc.declare_dram_parameter("kc8", [128, HPC * P], fp8, isOutput=False)
    vc8_d = nc.declare_dram_parameter("vc8", [128, HPC * P], i8, isOutput=False)
    colsb_d = nc.declare_dram_parameter("colsb", [128, NKC + 1], bf16,
                                        isOutput=False)
    out_d = nc.declare_dram_parameter("out", [n_iters, DIM // NCORES], f32,
                                      isOutput=True)
    if dbg:
        dbg_rows_d = nc.declare_dram_parameter("dbg_rows", [1, 4 * LOC], f32,
                                               isOutput=True)
        dbg_es_d = nc.declare_dram_parameter("dbg_es", [128, HPC * NTC], f32,
                                             isOutput=True)
        dbg_oc_d = nc.declare_dram_parameter("dbg_oc", [128, 2 * HPC], f32,
                                             isOutput=True)
        dbg_orow_d = nc.declare_dram_parameter("dbg_orow", [1, DIM], f32,
                                               isOutput=True)

    with tile.TileContext(nc) as tc:
        with (
            tc.tile_pool(name="sb", bufs=1) as sb,
            tc.tile_pool(name="wqp", bufs=2) as wqp,
            tc.tile_pool(name="wkvp", bufs=5) as wkvp,
            tc.tile_pool(name="wop", bufs=3) as wop,
            tc.tile_pool(name="vp", bufs=4) as vp,
            tc.tile_pool(name="psrow", bufs=3, space="PSUM") as psrow,
            tc.tile_pool(name="pscol", bufs=3, space="PSUM") as pscol,
            tc.tile_pool(name="psbig", bufs=2, space="PSUM") as psbig,
            tc.tile_pool(name="dram", bufs=1, space="DRAM") as dram,
        ):
            for _it in range(n_iters):
                # ---- all DMAs up front, in arrival order ----------------
                colsb = sb.tile([128, NKC + 1], bf16, tag="colsb")
                nc.sync.dma_start(colsb[:], colsb_d[:, :])

                wkvt = [None] * 6
                def wkv_dma(ch):
                    wt = wqp.tile([128, CW], bf16, tag="w")
                    nc.sync.dma_start(wt[:], wkv_d[:, ch * CW:(ch + 1) * CW])
                    wkvt[ch] = wt

                wkv_dma(0)
                rows = sb.tile([1, ROWS_LEN], f32, tag="rows")
                nc.sync.dma_start(rows[:], rows_d[:, :])
                wkv_dma(1)
                cols = sb.tile([128, COLS_W], f32, tag="cols")
                nc.sync.dma_start(cols[:], cols_d[:, :])
                one = rows[0:1, ONES:ONES + 1]

                kf8a = sb.tile([128, 2 * P], fp8, tag="kf8a")
                nc.sync.dma_start(kf8a[:], kc8_d[:, 0:2 * P])
                kf8b = sb.tile([128, 2 * P], fp8, tag="kf8b")
                nc.sync.dma_start(kf8b[:], kc8_d[:, 2 * P:4 * P])

                vi8 = []
                for h in range(HPC):
                    vi = vp.tile([128, P], i8, tag="vi8")
                    nc.sync.dma_start(vi[:], vc8_d[:, h * P:(h + 1) * P])
                    vi8.append(vi)

                wkvs = [None] * 8            # 1 MiB wk/wv chunks
                for i, qq in enumerate(range(8)):
                    # wk quarters then wv quarters
                    wt = wkvp.tile([128, CW // 2], bf16, tag="wkv1m")
                    base = 2 * CW + qq * (CW // 2)
                    nc.sync.dma_start(wt[:], wkv_d[:, base:base + CW // 2])
                    wkvs[qq] = wt

                wot = []                     # 3x1MiB + 2x0.5MiB wo chunks
                for wc in range(3):
                    wt = wop.tile([128, WOC], bf16, tag="w")
                    nc.sync.dma_start(wt[:], won_d[:, wc * WOC:(wc + 1) * WOC])
                    wot.append(wt)
                woh = []
                for hc in range(2):
                    wt = wop.tile([128, WOC // 2], bf16, tag="wh")
                    base = 3 * WOC + hc * (WOC // 2)
                    nc.sync.dma_start(wt[:], won_d[:, base:base + WOC // 2])
                    woh.append(wt)

                # ---- PE warm-up: ~3.5us dense burst so the PE clock
                # (HAM / sim pstate) reaches full rate before the real
                # matmul streams begin. Uses only the small "cols" tile.
                pwu = psrow.tile([1, 512], f32, tag="pw")
                wup = sb.tile([128, 256], bf16, tag="wup")  # scratch rhs
                nc.gpsimd.memset(wup[:], 0.0)
                for _w in range(48):
                    nc.tensor.matmul(pwu[0:1, 0:NKC + 1], colsb[:, 0:1],
                                     colsb[:, 0:NKC + 1],
                                     start=True, stop=True,
                                     skip_group_check=True)
                for _w in range(30):
                    nc.tensor.matmul(pwu[0:1, 0:256], colsb[:, 0:1], wup[:],
                                     start=True, stop=True,
                                     skip_group_check=True)

                # ---- q projection --------------------------------------
                psq = psrow.tile([1, 512], f32, tag="pw")
                for half in range(2):
                    for j in range(16):
                        kc = half * 16 + j
                        nc.tensor.matmul(
                            psq[:], colsb[:, kc:kc + 1],
                            wkvt[half][:, j * 512:(j + 1) * 512],
                            start=(kc == 0), stop=(kc == NKC - 1),
                        )

                tmp = sb.tile([1, 1024], f32, tag="tmp")

                def rope(dst, src, co, so):
                    e = src[0:1, 0:LOC:2]
                    o = src[0:1, 1:LOC:2]
                    c = rows[0:1, co:co + 256]
                    s = rows[0:1, so:so + 256]
                    nc.vector.tensor_mul(tmp[0:1, 0:256], e, c)
                    nc.vector.tensor_mul(tmp[0:1, 256:512], o, s)
                    nc.vector.tensor_sub(dst[0:1, 0:LOC:2], tmp[0:1, 0:256],
                                         tmp[0:1, 256:512])
                    nc.vector.tensor_mul(tmp[0:1, 512:768], e, s)
                    nc.vector.tensor_mul(tmp[0:1, 768:1024], o, c)
                    nc.vector.tensor_add(dst[0:1, 1:LOC:2], tmp[0:1, 512:768],
                                         tmp[0:1, 768:1024])

                q2 = sb.tile([1, LOC], f32, tag="q2")
                rope(q2, psq, QCOS, QSIN)
                q1 = sb.tile([1, LOC], f32, tag="q1")
                nc.vector.tensor_mul(q1[:], q2[:], rows[0:1, QS1:QS1 + LOC])
                pq1 = pscol.tile([128, HPC], f32, tag="pc")
                for h in range(HPC):
                    nc.tensor.matmul(pq1[:, h:h + 1],
                                     q1[0:1, h * DH:(h + 1) * DH],
                                     one, start=True, stop=True)
                q1c = sb.tile([128, HPC], bf16, tag="q1c")
                nc.vector.tensor_copy(q1c[:], pq1[:])
                pq2k = pscol.tile([128, 2 * HPC], f32, tag="pc")
                for h in range(HPC):
                    nc.tensor.matmul(pq2k[:, h:h + 1],
                                     q2[0:1, h * DH:(h + 1) * DH], one,
                                     start=True, stop=True)
                c8f = sb.tile([128, 2 * HPC], f32, tag="c8f")
                nc.vector.tensor_copy(c8f[:, 0:HPC], pq2k[:, 0:HPC])

                # ---- QK scores over the fp8 K cache --------------------
                s_all = psbig.tile([128, HPC * NTC], f32, tag="pb")
                es = sb.tile([128, HPC * NTC], bf16, tag="es")
                rs = sb.tile([128, HPC], f32, tag="rs")
                for h in range(HPC):
                    kf = kf8a if h < 2 else kf8b
                    hh = h % 2
                    for c in range(NTC):
                        nc.tensor.matmul(
                            s_all[:, h * NTC + c: h * NTC + c + 1],
                            kf[:, (hh * NTC + c) * 128:(hh * NTC + c + 1) * 128],
                            q1c[:, h:h + 1],
                            start=True, stop=True,
                        )
                    nc.scalar.activation(
                        es[:, h * NTC:(h + 1) * NTC],
                        s_all[:, h * NTC:(h + 1) * NTC],
                        AF.Exp,
                        accum_out=rs[:, h:h + 1],
                    )

                # past-token softmax denominators (only needs rs)
                psums = psrow.tile([1, 512], f32, tag="pw")
                nc.tensor.matmul(psums[0:1, 0:HPC], cols[:, ONESC:ONESC + 1],
                                 rs[:], start=True, stop=True)

                # ---- V int8 -> bf16 on idle compute engines -----------
                vfs = []
                for h in range(HPC):
                    vf = vp.tile([128, P], bf16, tag="kv")
                    nc.vector.tensor_copy(vf[:], vi8[h][:])
                    vfs.append(vf)

                # ---- PV over the bf16 V cache --------------------------
                po = pscol.tile([128, HPC], f32, tag="pc")
                for h in range(HPC):
                    for c in range(NTC):
                        nc.tensor.matmul(
                            po[:, h:h + 1],
                            vfs[h][:, c * 128:(c + 1) * 128],
                            es[:, h * NTC + c:h * NTC + c + 1],
                            start=(c == 0), stop=(c == NTC - 1),
                            skip_group_check=True,
                        )

                o1 = sb.tile([128, HPC], f32, tag="o1")
                nc.vector.tensor_mul(o1[:], po[:], cols[:, SCBV:SCBV + HPC])

                # ---- k/v projections (current token) -------------------
                def proj_kv(t):
                    ps = psrow.tile([1, 512], f32, tag="pw")
                    for ch in range(4):
                        wt = wkvs[4 * t + ch]
                        for j in range(8):
                            kc = ch * 8 + j
                            nc.tensor.matmul(
                                ps[:], colsb[:, kc:kc + 1],
                                wt[:, j * 512:(j + 1) * 512],
                                start=(kc == 0), stop=(kc == NKC - 1),
                            )
                    return ps

                psk = proj_kv(0)
                krot = sb.tile([1, LOC], f32, tag="krot")
                rope(krot, psk, KCOS, KSIN)

                # current-token score + softmax scale (off critical path)
                for h in range(HPC):
                    nc.tensor.matmul(
                        pq2k[:, HPC + h: HPC + h + 1],
                        krot[0:1, h * DH:(h + 1) * DH], one,
                        start=True, stop=True)
                nc.vector.tensor_copy(c8f[:, HPC:2 * HPC], pq2k[:, HPC:2 * HPC])
                pcur = psrow.tile([1, 512], f32, tag="pw")
                for h in range(HPC):
                    nc.tensor.matmul(
                        pcur[0:1, h:h + 1],
                        c8f[:, h:h + 1], c8f[:, HPC + h:HPC + h + 1],
                        start=True, stop=True)
                ecur = sb.tile([1, HPC], f32, tag="ec")
                nc.scalar.activation(ecur[:], pcur[0:1, 0:HPC], AF.Exp)
                tot = sb.tile([1, HPC], f32, tag="tot")
                nc.vector.tensor_add(tot[:], psums[0:1, 0:HPC], ecur[:])
                inv = sb.tile([1, HPC], f32, tag="inv")
                nc.vector.reciprocal(inv[:], tot[:])
                pb = pscol.tile([128, HPC], f32, tag="pc")
                nc.tensor.matmul(pb[:], rows[0:1, ONES:ONES + 128], inv[:],
                                 start=True, stop=True)
                invb = sb.tile([128, HPC], f32, tag="invb")
                nc.vector.tensor_copy(invb[:], pb[:])

                psv = proj_kv(1)
                vrow = sb.tile([1, LOC], f32, tag="vrow")
                nc.scalar.copy(vrow[:], psv[:])
                po2 = pscol.tile([128, HPC], f32, tag="pc")
                for h in range(HPC):
                    nc.tensor.matmul(
                        po2[:, h:h + 1],
                        vrow[0:1, h * DH:(h + 1) * DH],
                        ecur[0:1, h:h + 1],
                        start=True, stop=True,
                        skip_group_check=True,
                    )
                o2 = sb.tile([128, HPC], f32, tag="o2")
                nc.vector.tensor_add(o2[:], po2[:], o1[:])
                ocol = sb.tile([128, HPC], bf16, tag="ocol")
                nc.vector.tensor_mul(ocol[:], o2[:], invb[:])

                # ---- wo matvec (n-major chunks) ------------------------
                out_row = sb.tile([1, DIM], f32, tag="orow")
                for n in range(8):
                    if n < 6:
                        wt, nn = wot[n // 2], n % 2
                    else:
                        wt, nn = woh[n - 6], 0
                    pw = psrow.tile([1, 512], f32, tag="pw")
                    for ec in range(HPC):
                        nc.tensor.matmul(
                            pw[:],
                            ocol[:, ec:ec + 1],
                            wt[:, (nn * HPC + ec) * 512:
                                  (nn * HPC + ec + 1) * 512],
                            start=(ec == 0), stop=(ec == HPC - 1),
                        )
                    if n % 2 == 0:
                        nc.scalar.copy(out_row[0:1, n * 512:(n + 1) * 512],
                                       pw[:])
                    else:
                        nc.vector.tensor_copy(
                            out_row[0:1, n * 512:(n + 1) * 512], pw[:])

                # ---- ReduceScatter over 8 cores + output shard ---------
                if skip_rs:
                    nc.scalar.dma_start(out_d[_it:_it + 1, :],
                                        out_row[0:1, 0:DIM // NCORES])
                else:
                    cc_in = dram.tile([1, DIM], f32)
                    cc_out = dram.tile([1, DIM // NCORES], f32)
                    nc.scalar.dma_start(cc_in[0:1, 0:DIM // 2],
                                        out_row[0:1, 0:DIM // 2])
                    nc.scalar.dma_start(cc_in[0:1, DIM // 2:DIM],
                                        out_row[0:1, DIM // 2:DIM])
                    nc.gpsimd.collective_compute(
                        "ReduceScatter",
                        mybir.AluOpType.add,
                        ins=[cc_in.opt()],
                        outs=[cc_out.opt()],
                        replica_groups=[list(range(NCORES))],
                    )
                    nc.gpsimd.dma_start(out_d[_it:_it + 1, :], cc_out[:])

                if dbg:
                    nc.sync.dma_start(dbg_rows_d[0:1, 0:LOC], q1[:])
                    nc.sync.dma_start(dbg_rows_d[0:1, LOC:2 * LOC], q2[:])
                    nc.sync.dma_start(dbg_rows_d[0:1, 2 * LOC:3 * LOC], krot[:])
                    nc.sync.dma_start(dbg_rows_d[0:1, 3 * LOC:4 * LOC], vrow[:])
                    nc.gpsimd.dma_start(dbg_es_d[:, :], es[:])
                    dbgoc = sb.tile([128, 2 * HPC], f32, tag="dbgoc")
                    nc.vector.tensor_copy(dbgoc[:, 0:HPC], po[:])
                    nc.vector.tensor_copy(dbgoc[:, HPC:2 * HPC], o2[:])
                    nc.sync.dma_start(dbg_oc_d[:, :], dbgoc[:])
                    nc.sync.dma_start(dbg_orow_d[:, :], out_row[:])

    nc.finalize()
    return nc


def _prep_inputs(x, wq, wk, wv, wo, freqs_cos, freqs_sin, scb_k, scb_v,
                 cache_k_int8, cache_v_int8):
    """Build per-core in_maps (host-side sharding + layout)."""
    x = np.asarray(x, dtype=np.float32).reshape(DIM)
    fc = np.asarray(freqs_cos, dtype=np.float32).reshape(64)
    fs = np.asarray(freqs_sin, dtype=np.float32).reshape(64)
    scb_k = np.asarray(scb_k, dtype=np.float32).reshape(H, DH)
    scb_v = np.asarray(scb_v, dtype=np.float32).reshape(H, DH)
    kc = np.asarray(cache_k_int8).astype(np.float32).astype(FP8).reshape(H, DH, P)
    vc = np.asarray(cache_v_int8).astype(np.int8).reshape(H, DH, P)
    wq = np.asarray(wq, dtype=np.float32)
    wk = np.asarray(wk, dtype=np.float32)
    wv = np.asarray(wv, dtype=np.float32)
    wo = np.asarray(wo, dtype=np.float32)

    x_col = np.ascontiguousarray(x.reshape(NKC, 128).T)  # [128, 32]
    isq = 1.0 / np.sqrt(DH)

    in_maps = []
    for c in range(NCORES):
        hs = slice(c * HPC, (c + 1) * HPC)
        rsl = slice(c * LOC, (c + 1) * LOC)

        def pack_w(m):  # [512, 4096] -> [128, 32*512], chunk-interleaved
            return (m.T.reshape(NKC, 128, LOC).transpose(1, 0, 2)
                    .reshape(128, NKC * LOC))

        wkv_p = np.ascontiguousarray(np.concatenate(
            [pack_w(wq[rsl]), pack_w(wk[rsl]), pack_w(wv[rsl])],
            axis=1)).astype(BF16)

        # wo n-major: chunk wc covers out-blocks n in {2wc, 2wc+1}; within a
        # block, 4 e-chunks of [128, 512]
        wot = wo[:, rsl].T                       # [512(e), 4096(n)]
        won = (wot.reshape(HPC, 128, 8, 512)     # [ec, p, n, j]
               .transpose(1, 2, 0, 3)            # [p, n, ec, j]
               .reshape(128, HPC * DIM))
        won = np.ascontiguousarray(won).astype(BF16)

        kc8 = np.ascontiguousarray(
            kc[hs].transpose(1, 0, 2).reshape(128, HPC * P))
        # kc8[d, h*P + t] = K[h, d, t]; QK lhsT chunks are [d, t-chunk] blocks
        # vc8[p, h*P + c*128 + d] = V[h, d, c*128 + p]
        vc8 = np.ascontiguousarray(
            vc[hs].reshape(HPC, DH, NTC, 128).transpose(3, 0, 2, 1)
            .reshape(128, HPC * P))

        cols = np.zeros((128, COLS_W), dtype=np.float32)
        cols[:, XCOL:XCOL + NKC] = x_col
        cols[:, SCBV:SCBV + HPC] = scb_v[hs].T / 127.0
        cols[:, ONESC] = 1.0

        rows = np.zeros((1, ROWS_LEN), dtype=np.float32)
        rows[0, QCOS:QCOS + 256] = np.tile(fc, HPC) * isq
        rows[0, QSIN:QSIN + 256] = np.tile(fs, HPC) * isq
        rows[0, KCOS:KCOS + 256] = np.tile(fc, HPC)
        rows[0, KSIN:KSIN + 256] = np.tile(fs, HPC)
        rows[0, QS1:QS1 + LOC] = scb_k[hs].reshape(LOC) / 127.0
        rows[0, ONES:ONES + 128] = 1.0

        colsb = np.zeros((128, NKC + 1), dtype=BF16)
        colsb[:, 0:NKC] = x_col.astype(BF16)
        colsb[:, NKC] = BF16(1.0)
        in_maps.append(dict(cols=cols, rows=rows, wkv=wkv_p,
                            won=won, kc8=kc8, vc8=vc8, colsb=colsb))
    return in_maps


def kernel(x, wq, wk, wv, wo, freqs_cos, freqs_sin, scb_k, scb_v,
           cache_k_int8, cache_v_int8, start_pos=P, **_ignored):
    from concourse.bass_utils import run_bass_kernel_spmd

    assert int(start_pos) == P, f"kernel hardcodes start_pos={P}"
    if "nc" not in _CACHE:
        _CACHE["nc"] = _build_nc()
    nc = _CACHE["nc"]

    in_maps = _prep_inputs(x, wq, wk, wv, wo, freqs_cos, freqs_sin,
                           scb_k, scb_v, cache_k_int8, cache_v_int8)
    res = run_bass_kernel_spmd(nc, in_maps, core_ids=list(range(NCORES)))
    out = np.concatenate(
        [np.asarray(res.results[c]["out"], dtype=np.float32)
         .reshape(-1)[:DIM // NCORES] for c in range(NCORES)])
    return out.reshape(1, 1, DIM)

